# revision 40
# baseline (speedup 1.0000x reference)
"""AnchorAttention Trainium2 kernel, SPMD over 8 NeuronCores.

Problem (hardcoded shapes): x (4, 4096, 1024) f32; every token attends to
the A=512 anchor tokens only.  H=16 heads, D=64 head dim.
  qkv   = x[:, :A] @ Wqkv + bqkv        (anchor K/V, and anchor Q)
  q     = x[:, A:] @ Wq + bq            (query-token Q)
  out_h = softmax(q_h @ k_h^T / 8) @ v_h
  y     = concat_h(out) @ Wproj + bproj

Sharding: core i -> (batch b = i//2, token half j = i%2, 2048 tokens).
Each core computes its tokens end to end; anchor K/V are recomputed per
core pair.  No collectives.

Flat software-pipelined schedule (vs the v1 per-block phase structure,
which serialized PE behind the ScalarE exp stream in each block's
scores/AV middle and spent ~15us in rank-1 bias matmuls + ~17us in
blind warm-up).  One stream of 32 per-head-pair "units"; unit u for
pair (b, i) emits, in order:
  [KT group i+2 (b0 only)] [scores a=0,1] [Qproj group of block b+1]
  [scores a=2,3] [AV even+odd of pair u-1] [proj group u-9] [mul-odd]
so TensorE always has dense independent work while ACT grinds exps and
DVE runs the softmax-normalize chain.  Key scheduling rules learned on
hardware:
  - proj lags one full unit (u-9): proj(b, 0) must not wait on the
    same-unit rb-DMA -> mul chain of pair (b, 7);
  - ScalarE stays pure-exp (evacs behind 8 exps in the ACT FIFO free
    PSUM too late for the 2-buffer accumulation pool);
  - KT needs a 2-unit lead or scores stall on a DVE-deep evacuation;
  - V-group evacuations are staggered per group, else the end-of-front
    pile stalls unit 0 ~3.6us and HAM re-throttles the PE clock;
  - PSUM = 6-bank ring (scores + AV share it) + 2 accumulation banks;
  - per-partition q/k biases fold into the DVE PSUM-evacuation
    (tensor_scalar); all bias work compiles out when the runtime
    detects all-zero biases (zero_bias build variant).
"""

import sys
from contextlib import ExitStack

sys.path.insert(0, "/opt/trn_rl_repo")

import ml_dtypes
import numpy as np

import concourse.bass as bass
import concourse.mybir as mybir
import concourse.tile as tile
from concourse import bacc
from concourse.bass_utils import run_bass_kernel_spmd

F32 = mybir.dt.float32
BF16 = mybir.dt.bfloat16

B, S, DIM = 4, 4096, 1024
H, D = 16, 64
A = 512              # anchor tokens
TOK = 2048           # tokens per core
NBLK = 4             # 512-token blocks per core
BLK = 512
N_CORES = 8
SCALE = 1.0 / np.sqrt(D)

KQ = DIM // 128      # 8 qk-dim tiles
KD = DIM // 128      # 8 contraction tiles
NA = A // 128        # 4 anchor tiles
NP = H // 2          # 8 head pairs per block

_COMPILED = {}


def build_kernel(zero_bias=False):
    nc = bacc.Bacc(trn_type="TRN2", target_bir_lowering=False)

    xT = nc.declare_dram_parameter("xT", [DIM, TOK], BF16, isOutput=False)
    aT = nc.declare_dram_parameter("aT", [DIM, A], BF16, isOutput=False)
    wkv = nc.declare_dram_parameter("wkv", [DIM, 2 * DIM], BF16, isOutput=False)
    wqa = nc.declare_dram_parameter("wqa", [DIM, DIM], BF16, isOutput=False)
    wqb = nc.declare_dram_parameter("wqb", [DIM, DIM], BF16, isOutput=False)
    wproj = nc.declare_dram_parameter("wproj", [DIM, DIM], BF16, isOutput=False)
    # per-partition bias tiles: [:, m] = bias[m*128:(m+1)*128]
    bkT = nc.declare_dram_parameter("bkT", [128, KQ], F32, isOutput=False)
    bqaT = nc.declare_dram_parameter("bqaT", [128, KQ], F32, isOutput=False)
    bqbT = nc.declare_dram_parameter("bqbT", [128, KQ], F32, isOutput=False)
    bv_row = nc.declare_dram_parameter("bv_row", [1, DIM], BF16, isOutput=False)
    y = nc.declare_dram_parameter("y", [TOK, DIM], BF16, isOutput=True)

    with tile.TileContext(nc) as tc, ExitStack() as ctx:
        const = ctx.enter_context(tc.tile_pool(name="const", bufs=1))
        p_w = ctx.enter_context(tc.tile_pool(name="p_w", bufs=1))
        p_kt = ctx.enter_context(tc.tile_pool(name="p_kt", bufs=1))
        p_v = ctx.enter_context(tc.tile_pool(name="p_v", bufs=1))
        p_at = ctx.enter_context(tc.tile_pool(name="p_at", bufs=1))
        p_xt = ctx.enter_context(tc.tile_pool(name="p_xt", bufs=16))
        p_qt = ctx.enter_context(tc.tile_pool(name="p_qt", bufs=18))
        p_exp = ctx.enter_context(tc.tile_pool(name="p_exp", bufs=18))
        p_rb = ctx.enter_context(tc.tile_pool(name="p_rb", bufs=3))
        p_den = ctx.enter_context(tc.tile_pool(name="p_den", bufs=3))
        p_ot = ctx.enter_context(tc.tile_pool(name="p_ot", bufs=16))
        p_y = ctx.enter_context(tc.tile_pool(name="p_y", bufs=4))
        p_pa = ctx.enter_context(tc.tile_pool(name="p_pa", bufs=8))
        p_ring = ctx.enter_context(tc.tile_pool(name="p_ring", bufs=6, space="PSUM"))
        p_mm = ctx.enter_context(tc.tile_pool(name="p_mm", bufs=2, space="PSUM"))

        def ring_tile(name):
            return p_ring.tile([128, BLK], F32, tag="ring", name=name)

        def mm_tile(name):
            return p_mm.tile([128, BLK], F32, tag="mm", name=name)

        def psum8(name):
            """8 simultaneous banks for a k-outer accumulation phase."""
            return [ring_tile(f"{name}{m}") for m in range(6)] + [
                mm_tile(f"{name}{m}") for m in (6, 7)
            ]

        # ---- tiny constants + exp-table preload ----
        ones_row = const.tile([1, 128], BF16, tag="ones_row")
        nc.any.memset(ones_row, 1.0)
        dummy = const.tile([1, 16], F32, tag="dummy")
        nc.vector.memset(dummy, 0.0)
        dummy2 = const.tile([1, 16], BF16, tag="dummy2")
        # preload the exp table set (~2.7us) during the DMA front
        nc.scalar.activation(dummy2[:], dummy[:], mybir.ActivationFunctionType.Exp)

        bk_sb = const.tile([128, KQ], F32, tag="bk_sb")
        nc.sync.dma_start(bk_sb[:], bkT[:, :])
        bqa_sb = const.tile([128, KQ], F32, tag="bqa_sb")
        nc.sync.dma_start(bqa_sb[:], bqaT[:, :])
        bqb_sb = const.tile([128, KQ], F32, tag="bqb_sb")
        nc.sync.dma_start(bqb_sb[:], bqbT[:, :])
        bv_sb = const.tile([1, DIM], BF16, tag="bv_sb")
        nc.sync.dma_start(bv_sb[:], bv_row[:, :])

        # ---- input DMAs, k-tile interleaved so compute starts early ----
        xt_sb = [[None] * KD for _ in range(NBLK)]
        wqa_sb, wqb_sb, wkv_sb, wp_sb, at_sb = [], [], [], [], []

        def dma_xt(b, k):
            t = p_xt.tile([128, BLK], BF16, tag="xt", name=f"xt{b}_{k}")
            nc.sync.dma_start(
                t[:], xT[128 * k : 128 * (k + 1), BLK * b : BLK * (b + 1)]
            )
            xt_sb[b][k] = t

        for k in range(KD):
            dma_xt(0, k)
            t = p_w.tile([128, DIM], BF16, name=f"wqa{k}", tag=f"wqa{k}")
            nc.sync.dma_start(t[:], wqa[128 * k : 128 * (k + 1), :])
            wqa_sb.append(t)
        for k in range(KD):
            t = p_at.tile([128, A], BF16, name=f"at{k}", tag=f"at{k}")
            nc.sync.dma_start(t[:], aT[128 * k : 128 * (k + 1), :])
            at_sb.append(t)
            t = p_w.tile([128, 2 * DIM], BF16, name=f"wkv{k}", tag=f"wkv{k}")
            nc.sync.dma_start(t[:], wkv[128 * k : 128 * (k + 1), :])
            wkv_sb.append(t)
        for k in range(KD):
            dma_xt(1, k)
            t = p_w.tile([128, DIM], BF16, name=f"wqb{k}", tag=f"wqb{k}")
            nc.sync.dma_start(t[:], wqb[128 * k : 128 * (k + 1), :])
            wqb_sb.append(t)
        for k in range(KD):
            t = p_w.tile([128, DIM], BF16, name=f"wp{k}", tag=f"wp{k}")
            nc.sync.dma_start(t[:], wproj[128 * k : 128 * (k + 1), :])
            wp_sb.append(t)

        # ---- PE warm-up until the first xt/wqa tiles land (~3us) ----
        warm = const.tile([128, 256], BF16, tag="warm")
        nc.vector.memset(warm[:], 0.0)
        wps = mm_tile("warmps")
        for i in range(38):
            nc.tensor.matmul(
                wps[:, 0:256], warm[:, 0:128], warm[:],
                start=True, stop=True,
            )

        qt_sb = [[None] * KQ for _ in range(NBLK)]

        def emit_qproj_group(b, m):
            """One Qproj output group: qt[b][m] = (Wq^T xT)[m] + bq[m].
            Evacuation on DVE: ScalarE must stay pure-exp — an evac behind
            the unit's 8 exps in the ACT FIFO releases the PSUM bank too
            late for the 2-buffer mm pool."""
            wq = wqa_sb if b == 0 else wqb_sb
            bq = bqa_sb if b == 0 else bqb_sb
            ps = mm_tile(f"qps{b}_{m}")
            for k in range(KD):
                nc.tensor.matmul(
                    ps[:], wq[k][:, 128 * m : 128 * (m + 1)], xt_sb[b][k][:],
                    start=(k == 0), stop=(k == KD - 1),
                )
            qt = p_qt.tile([128, BLK], BF16, tag="qt", name=f"qt{b}_{m}")
            if zero_bias:
                nc.vector.tensor_copy(qt[:], ps[:])
            else:
                nc.vector.tensor_scalar_add(qt[:], ps[:], bq[:, m : m + 1])
            qt_sb[b][m] = qt

        # ---- front: Qproj(b0) k-outer (DMA-paced), then KT, then V ----
        qps = psum8("qps0_")
        for k in range(KD):
            for m in range(KQ):
                nc.tensor.matmul(
                    qps[m][:], wqa_sb[k][:, 128 * m : 128 * (m + 1)],
                    xt_sb[0][k][:],
                    start=(k == 0), stop=(k == KD - 1),
                )
        for m in range(KQ):
            qt = p_qt.tile([128, BLK], BF16, tag="qt", name=f"qt0_{m}")
            if zero_bias:
                nc.vector.tensor_copy(qt[:], qps[m][:])
            else:
                nc.vector.tensor_scalar_add(qt[:], qps[m][:], bqa_sb[:, m : m + 1])
            qt_sb[0][m] = qt

        # KT[qk, a] = Wk^T aT (+bk): group m is emitted inside unit m
        # (scores of pair i only need kt[i]); this keeps the front short.
        kt_sb = [None] * KQ

        def emit_kt_group(m):
            ps = mm_tile(f"kps_{m}")
            for k in range(KD):
                nc.tensor.matmul(
                    ps[:], wkv_sb[k][:, 128 * m : 128 * (m + 1)], at_sb[k][:],
                    start=(k == 0), stop=(k == KD - 1),
                )
            kt = p_kt.tile([128, A], BF16, name=f"kt{m}", tag=f"kt{m}")
            if zero_bias:
                nc.vector.tensor_copy(kt[:], ps[:])
            else:
                nc.vector.tensor_scalar_add(kt[:], ps[:], bk_sb[:, m : m + 1])
            kt_sb[m] = kt

        # V tiles: per head pair [V_even | ones | ones | V_odd], k-outer
        v_sb = []
        for a in range(NA):
            t = p_v.tile([128, 2 * DIM], BF16, name=f"v{a}", tag=f"v{a}")
            nc.vector.memset(
                t[:].rearrange("p (hp c) -> p hp c", c=4 * D)[:, :, D : 3 * D], 1.0
            )
            v_sb.append(t)
        # k-outer for k=0..6, then per-group [k=7, bias, evac] so PSUM banks
        # free incrementally — a single end-of-phase evac pile stalled the
        # first unit's KT group ~3.6us and re-throttled HAM
        vps = psum8("vps_")
        for k in range(KD - 1):
            for g in range(8):
                n, a = g // 4, g % 4
                nc.tensor.matmul(
                    vps[g][:],
                    at_sb[k][:, 128 * a : 128 * (a + 1)],
                    wkv_sb[k][:, DIM + 512 * n : DIM + 512 * (n + 1)],
                    start=(k == 0), stop=False,
                )
        # mm-pool groups (6, 7) evacuate FIRST — KT(0)/KT(1) reuse those
        # banks and otherwise stall behind the whole evacuation queue long
        # enough to re-throttle HAM; the second copy of each pair goes to
        # ScalarE (idle during the front) to halve the DVE pile
        for g in (6, 7, 0, 1, 2, 3, 4, 5):
            n, a = g // 4, g % 4
            nc.tensor.matmul(
                vps[g][:],
                at_sb[KD - 1][:, 128 * a : 128 * (a + 1)],
                wkv_sb[KD - 1][:, DIM + 512 * n : DIM + 512 * (n + 1)],
                start=False, stop=zero_bias,
            )
            if not zero_bias:
                nc.tensor.matmul(
                    vps[g][:], ones_row[:], bv_sb[:, 512 * n : 512 * (n + 1)],
                    start=False, stop=True,
                )
            vr = v_sb[a][:].rearrange("p (hp c) -> p hp c", c=4 * D)
            pr = vps[g][:].rearrange("p (hp c) -> p hp c", c=2 * D)
            nc.vector.tensor_copy(vr[:, 4 * n : 4 * (n + 1), 0:D], pr[:, :, 0:D])
            nc.scalar.copy(
                vr[:, 4 * n : 4 * (n + 1), 3 * D : 4 * D], pr[:, :, D : 2 * D]
            )

        # KT for the first two pairs up front; units emit KT(i+2) so the
        # scores of pair i never wait on a same-unit KT evacuation
        emit_kt_group(0)
        emit_kt_group(1)

        # ---- steady-state unit stream ----
        ot_sb = [[None] * KQ for _ in range(NBLK)]

        def emit_score_slot(b, i, a):
            """Row-packed scores for heads (2i, 2i+1), anchor tile a."""
            pse = ring_tile(f"pse{b}_{i}_{a}")
            nc.tensor.matmul(
                pse[:], kt_sb[i][0:D, 128 * a : 128 * (a + 1)], qt_sb[b][i][0:D, :],
                start=True, stop=True, tile_position=(0, 0),
            )
            pso = ring_tile(f"pso{b}_{i}_{a}")
            nc.tensor.matmul(
                pso[:],
                kt_sb[i][D : 2 * D, 128 * a : 128 * (a + 1)],
                qt_sb[b][i][D : 2 * D, :],
                start=True, stop=True, tile_position=(64, 0),
            )
            ee = p_exp.tile([128, BLK], BF16, tag="exp", name=f"ee{b}_{i}_{a}")
            nc.scalar.activation(
                ee[:], pse[:], mybir.ActivationFunctionType.Exp, scale=float(SCALE)
            )
            eo = p_exp.tile([128, BLK], BF16, tag="exp", name=f"eo{b}_{i}_{a}")
            nc.scalar.activation(
                eo[:], pso[:], mybir.ActivationFunctionType.Exp, scale=float(SCALE)
            )
            return ee, eo

        def emit_av_mm(b, i, h, e_tiles, name):
            """AV accumulation for head h of pair i; returns the psum tile."""
            av = ring_tile(name)
            for a in range(NA):
                nc.tensor.matmul(
                    av[:],
                    v_sb[a][:, 2 * D * h : 2 * D * (h + 1)],
                    e_tiles[a][:],
                    start=(a == 0), stop=(a == NA - 1),
                )
            return av

        def emit_av_even_evac1(b, i, av):
            """Even head: den rows live at [D:2D]; copy out (on GpSimd — DVE
            queue relief is via the ScalarE qt evac; GPSIMD cannot read
            PSUM) + DMA-shift to partition base 0."""
            den = p_den.tile([128, BLK], F32, tag="den", name=f"den{b}_{i}")
            nc.vector.tensor_copy(den[D : 2 * D, :], av[D : 2 * D, :])
            nc.sync.dma_start(den[0:D, :], den[D : 2 * D, :])
            return den

        def emit_av_even_evac2(b, i, av, den):
            rb = p_rb.tile([128, BLK], F32, tag="rb", name=f"rbe{b}_{i}")
            nc.vector.reciprocal_approx_fast(rb[0:D, :], den[0:D, :])
            nc.vector.tensor_mul(ot_sb[b][i][0:D, :], av[0:D, :], rb[0:D, :])

        def emit_av_odd_evac1(b, i, av):
            rb = p_rb.tile([128, BLK], F32, tag="rb", name=f"rbo{b}_{i}")
            nc.vector.reciprocal_approx_fast(rb[0:D, :], av[0:D, :])
            nc.sync.dma_start(rb[D : 2 * D, :], rb[0:D, :])
            return rb

        def emit_av_odd_evac2(b, i, av, rb):
            nc.vector.tensor_mul(
                ot_sb[b][i][D : 2 * D, :], av[D : 2 * D, :], rb[D : 2 * D, :]
            )

        def emit_proj_group(b, pg):
            """Proj output group pg (token tile tt = pg//2, dim half n = pg%2)."""
            tt, n = pg // 2, pg % 2
            csl = slice(128 * tt, 128 * (tt + 1))
            ps = mm_tile(f"pps{b}_{pg}")
            for k in range(KD):
                nc.tensor.matmul(
                    ps[:],
                    ot_sb[b][k][:, csl],
                    wp_sb[k][:, 512 * n : 512 * (n + 1)],
                    start=(k == 0), stop=(k == KD - 1),
                )
            yt = p_y.tile([128, 512], BF16, tag="y", name=f"y{b}_{pg}")
            nc.vector.tensor_copy(yt[:], ps[:])
            nc.sync.dma_start(
                y[
                    BLK * b + 128 * tt : BLK * b + 128 * (tt + 1),
                    512 * n : 512 * (n + 1),
                ],
                yt[:],
            )

        pa_sb = [None] * 8

        def emit_projA(pg):
            """proj(3, pg) partial over k=0..3 -> SBUF bf16 stash."""
            tt, n = pg // 2, pg % 2
            csl = slice(128 * tt, 128 * (tt + 1))
            ps = mm_tile(f"paps{pg}")
            for k in range(4):
                nc.tensor.matmul(
                    ps[:],
                    ot_sb[3][k][:, csl],
                    wp_sb[k][:, 512 * n : 512 * (n + 1)],
                    start=(k == 0), stop=(k == 3),
                )
            pa = p_pa.tile([128, 512], BF16, tag="pa", name=f"pa{pg}")
            nc.vector.tensor_copy(pa[:], ps[:])
            pa_sb[pg] = pa

        def emit_projB_pre(pg):
            """proj(3, pg) k=4..6 (no ot[3][7] dependency)."""
            tt, n = pg // 2, pg % 2
            csl = slice(128 * tt, 128 * (tt + 1))
            ps = mm_tile(f"pbps{pg}")
            for k in range(4, 7):
                nc.tensor.matmul(
                    ps[:],
                    ot_sb[3][k][:, csl],
                    wp_sb[k][:, 512 * n : 512 * (n + 1)],
                    start=(k == 4), stop=False,
                )
            return ps

        def emit_projB_fin(pg, ps):
            """k=7 close + the stashed k=0..3 partial -> y."""
            tt, n = pg // 2, pg % 2
            csl = slice(128 * tt, 128 * (tt + 1))
            nc.tensor.matmul(
                ps[:],
                ot_sb[3][7][:, csl],
                wp_sb[7][:, 512 * n : 512 * (n + 1)],
                start=False, stop=True,
            )
            yt = p_y.tile([128, 512], BF16, tag="y", name=f"yB{pg}")
            nc.vector.tensor_add(yt[:], ps[:], pa_sb[pg][:])
            nc.sync.dma_start(
                y[
                    BLK * 3 + 128 * tt : BLK * 3 + 128 * (tt + 1),
                    512 * n : 512 * (n + 1),
                ],
                yt[:],
            )

        # flat list of (b, i) pairs; AV of pair u runs inside unit u+1
        pairs = [(b, i) for b in range(NBLK) for i in range(NP)]

        def emit_unit(u):
            b, i = pairs[u]
            prev = pairs[u - 1] if u > 0 else None
            # KT group with a two-unit lead over the scores that use it
            if b == 0 and i < NP - 2:
                emit_kt_group(i + 2)
            if prev is not None:
                pb, pi = prev
                e_even = exp_tiles[prev][0]
                e_odd = exp_tiles[prev][1]
                ot_sb[pb][pi] = p_ot.tile(
                    [128, BLK], BF16, tag="ot", name=f"ot{pb}_{pi}"
                )

            # scores a=0,1 (+ exps)
            cur_e, cur_o = [], []
            for a in (0, 1):
                ee, eo = emit_score_slot(b, i, a)
                cur_e.append(ee)
                cur_o.append(eo)

            # Qproj AND proj quota between the score halves: the separator
            # gives the a01 exps time before their ring slots are reused by
            # a23 — without it (b3 units have no qproj) pse3/pso3 stall ~1us
            # on WARs against the same unit's exps
            if b == 0:
                emit_qproj_group(1, i)
            elif b == 1:
                emit_qproj_group(2, i)
            elif b == 2:
                emit_qproj_group(3, i)
            else:
                # b3 has no qproj: proj/projA serve as the separator here
                if u >= 9:
                    emit_proj_group((u - 9) // 8, (u - 9) % 8)
                if u == 29:
                    for pg in (0, 1):
                        emit_projA(pg)
                elif u == 30:
                    for pg in (2, 3):
                        emit_projA(pg)
                elif u == 31:
                    emit_projA(4)

            # scores a=2,3
            for a in (2, 3):
                ee, eo = emit_score_slot(b, i, a)
                cur_e.append(ee)
                cur_o.append(eo)
            exp_tiles[(b, i)] = (cur_e, cur_o)

            # AV of the previous pair, even then odd back-to-back
            if prev is not None:
                av_e = emit_av_mm(pb, pi, 2 * pi, e_even, f"ave{pb}_{pi}")
                den = emit_av_even_evac1(pb, pi, av_e)
                av_o = emit_av_mm(pb, pi, 2 * pi + 1, e_odd, f"avo{pb}_{pi}")
                rb_o = emit_av_odd_evac1(pb, pi, av_o)
                emit_av_even_evac2(pb, pi, av_e, den)

            # proj quota (ONE-UNIT lag) after AV for b0-b2: between the
            # score halves it congests the mm-pool evacuation cycle
            if b < 3 and u >= 9:
                emit_proj_group((u - 9) // 8, (u - 9) % 8)

            if prev is not None:
                emit_av_odd_evac2(pb, pi, av_o, rb_o)

            # JIT xt DMAs for block b+2
            if b == 0:
                dma_xt(2, i)
            elif b == 1:
                dma_xt(3, i)

        exp_tiles = {}
        for u in range(len(pairs)):
            emit_unit(u)

        # tail: AV of the last pair; the dep-free proj(2,7) fills the PE
        # while the evac chains drain; then the k=4..7 halves of proj(3)
        pb, pi = pairs[-1]
        e_even, e_odd = exp_tiles[(pb, pi)]
        ot_sb[pb][pi] = p_ot.tile([128, BLK], BF16, tag="ot", name=f"ot{pb}_{pi}")
        av_e = emit_av_mm(pb, pi, 2 * pi, e_even, f"ave{pb}_{pi}")
        av_o = emit_av_mm(pb, pi, 2 * pi + 1, e_odd, f"avo{pb}_{pi}")
        rb_o = emit_av_odd_evac1(pb, pi, av_o)
        # Even head, tail only: recompute the denominator via the ones
        # columns (256*pi+64 : 256*pi+192 are all ones -> den replicated on
        # every partition), so recip runs at base 0 with no COPY+DMA shift.
        # The shift's queue latency (behind draining y DMAs) idled the PE
        # >3.4us here and re-throttled HAM.
        dps = ring_tile("dupden")
        for a in range(NA):
            nc.tensor.matmul(
                dps[:], v_sb[a][:, 256 * pi + 64 : 256 * pi + 192],
                e_even[a][:],
                start=(a == 0), stop=(a == NA - 1),
            )
        rb_e = p_rb.tile([128, BLK], F32, tag="rb", name="rbe_tail")
        nc.vector.reciprocal_approx_fast(rb_e[0:D, :], dps[0:D, :])
        nc.vector.tensor_mul(
            ot_sb[pb][pi][0:D, :], av_e[0:D, :], rb_e[0:D, :]
        )
        emit_projA(5)
        emit_projA(6)
        emit_projA(7)
        emit_proj_group(2, 7)
        ps0 = emit_projB_pre(0)
        ps1 = emit_projB_pre(1)
        emit_av_odd_evac2(pb, pi, av_o, rb_o)
        emit_projB_fin(0, ps0)
        emit_projB_fin(1, ps1)
        for pg in range(2, 8):
            ps = emit_projB_pre(pg)
            emit_projB_fin(pg, ps)

    nc.compile()
    return nc


def _shard_inputs(x, Wqkv, bqkv, Wq, bq, Wproj):
    """Build the 8 per-core input dicts (host-side layout prep)."""
    x = np.asarray(x, dtype=np.float32)
    Wqkv = np.asarray(Wqkv, dtype=np.float32)
    bqkv = np.asarray(bqkv, dtype=np.float32)
    Wq = np.asarray(Wq, dtype=np.float32)
    bq = np.asarray(bq, dtype=np.float32)
    Wproj = np.asarray(Wproj, dtype=np.float32)

    bf16 = ml_dtypes.bfloat16
    wkv = np.ascontiguousarray(Wqkv[:, DIM:]).astype(bf16)
    wq_anchor = np.ascontiguousarray(Wqkv[:, :DIM]).astype(bf16)
    wq_b = Wq.astype(bf16)
    wproj_b = Wproj.astype(bf16)

    def bias_T(v):
        return np.ascontiguousarray(v.reshape(KQ, 128).T).astype(np.float32)

    bkT = bias_T(bqkv[DIM : 2 * DIM])
    bqaT_anchor = bias_T(bqkv[:DIM])
    bqbT = bias_T(bq)
    bv_row = np.ascontiguousarray(bqkv[2 * DIM :].reshape(1, DIM)).astype(bf16)

    in_maps = []
    for core in range(N_CORES):
        b, j = core // 2, core % 2
        xs = x[b, j * TOK : (j + 1) * TOK, :]
        in_maps.append(
            {
                "xT": np.ascontiguousarray(xs.T).astype(bf16),
                "aT": np.ascontiguousarray(x[b, :A, :].T).astype(bf16),
                "wkv": wkv,
                "wqa": wq_anchor if j == 0 else wq_b,
                "wqb": wq_b,
                "wproj": wproj_b,
                "bkT": bkT,
                "bqaT": bqaT_anchor if j == 0 else bqbT,
                "bqbT": bqbT,
                "bv_row": bv_row,
            }
        )
    return in_maps


def kernel(x, Wqkv, bqkv, Wq, bq, Wproj, bproj, num_anchor_tokens, **run_kwargs):
    assert int(num_anchor_tokens) == A
    zero_bias = bool(
        not np.any(np.asarray(bqkv)) and not np.any(np.asarray(bq))
    )
    key = ("nc", zero_bias)
    if key not in _COMPILED:
        _COMPILED[key] = build_kernel(zero_bias=zero_bias)
    nc = _COMPILED[key]
    in_maps = _shard_inputs(x, Wqkv, bqkv, Wq, bq, Wproj)
    res = run_bass_kernel_spmd(
        nc, in_maps, core_ids=list(range(N_CORES)), **run_kwargs
    )
    bproj = np.asarray(bproj, dtype=np.float32)
    out = np.empty((B, S, DIM), dtype=np.float32)
    for core in range(N_CORES):
        b, j = core // 2, core % 2
        out[b, j * TOK : (j + 1) * TOK, :] = np.asarray(
            res.results[core]["y"], dtype=np.float32
        )
    out += bproj[None, None, :]
    _COMPILED["last_result"] = res
    return out
